# revision 27
# baseline (speedup 1.0000x reference)
"""Trainium2 Bass kernel for nn_AttentionConv2D (two conv3x3+BN branches with
position-aware attention maps), SPMD over 8 NeuronCores.

Sharding: core = batch_index * 2 + h_half. Each core computes both branches for
one batch element's 128-row horizontal slab (plus 1-row halo for the attention
3x3 conv). All cross-core data movement is done host-side (overlapping input
row slices, per-core position/band constants); the device program is identical
on every core.

Device dataflow per core (f-row coords fr in [0,130), output rows fr in [1,129)):
  conv3x3 (both branches, 128 out ch) -> implicit GEMM, 9 taps x row-pair
    matmuls (fp32r, N=512) accumulated in PSUM, processed in 4-row groups
  f = psum * scale_c + bias_c           (ScalarE, evac)
  att = A^T @ f                         (PE, M=2 matmul; BN/scale folded into A)
  att row-layout repack                 (DMA)
  z2 = banded-matrix matmuls over rows  (PE, K=18 -> M=16 blocks, 3 x-shifts)
  map = sigmoid(z2 + pos2)              (VectorE add + ScalarE sigmoid)
  map broadcast to 128 channel partitions (DMA to partitions 0/32/64/96 +
    VectorE stream_shuffle, per 16-row block)
  out = f * map                         (VectorE, in place) -> DMA out
"""

import sys
from contextlib import ExitStack

import numpy as np

for _p in ("/opt/trn_rl_repo", "/root/.axon_site/_ro/trn_rl_repo"):
    if _p not in sys.path:
        sys.path.append(_p)

import concourse.bass as bass
import concourse.mybir as mybir
import concourse.tile as tile
from concourse.bass_utils import run_bass_kernel_spmd

F32 = mybir.dt.float32
F32R = mybir.dt.float32r
AF = mybir.ActivationFunctionType

# Problem constants (hardcoded per contract).
B, CIN, COUT, H, W = 4, 128, 128, 256, 256
BR = 64
EPS_BR = 1e-3
EPS_ATT = 1e-5
WP = W + 2            # padded row stride in SBUF
NF = 130              # f rows per core (128 + 1 halo each side)
G = 4                 # f rows per conv group
NGRP = 33             # 32 full groups + 1 tail group of 2 rows
XROWS = 12            # f rows per x input tile (3 conv groups)
NXT = 11              # number of x tiles
# attention blocks: (first out f-row, width). Last two are narrow so the
# final serial chain (sigmoid/shuffle/mul/out) is short.
BLOCKS = [(1, 16), (17, 16), (33, 16), (49, 16), (65, 16), (81, 16),
          (97, 16), (113, 8), (121, 8)]
NBLK = len(BLOCKS)
BW = 16               # max block width (tile sizing)
BK = 2 * BW + 2       # att_rt partition dim: bw+2 att rows + bw pos2 rows


def _grp_rows(g):
    """(start_f_row, n_rows) of conv group g."""
    return (G * g, 2 if g == NGRP - 1 else G)


def _band_off(b):
    """Column offset of block b's strip in the band matrix."""
    return sum(6 * bw for _, bw in BLOCKS[:b])


def emit_core(tc, outs, ins):
    """Emit the per-core program. outs/ins are dicts of DRAM APs."""
    nc = tc.nc
    out_d = outs["out"]
    xh_d, wf_d = ins["xh"], ins["wf"]
    chs_d, chb_d, av_d = ins["chs"], ins["chb"], ins["av"]
    band_d, pos2_d, zeros_d = ins["band"], ins["pos2"], ins["zeros"]

    ctx = ExitStack()
    with ctx:
        const = ctx.enter_context(tc.tile_pool(name="const", bufs=1))
        xp = ctx.enter_context(tc.tile_pool(name="xp", bufs=3))
        fbp = ctx.enter_context(tc.tile_pool(name="fbp", bufs=5))
        attp = ctx.enter_context(tc.tile_pool(name="attp", bufs=3))
        rtp = ctx.enter_context(tc.tile_pool(name="rtp", bufs=2))
        mapp = ctx.enter_context(tc.tile_pool(name="mapp", bufs=3))
        fps = ctx.enter_context(tc.tile_pool(name="fps", bufs=2, space="PSUM"))
        atps = ctx.enter_context(tc.tile_pool(name="atps", bufs=1, space="PSUM"))
        z2ps = ctx.enter_context(tc.tile_pool(name="z2ps", bufs=2, space="PSUM"))

        # Constants
        wf_sb = const.tile([CIN, 9 * COUT], F32R)
        nc.scalar.dma_start(wf_sb[:], wf_d[:].bitcast(F32R))
        chs_sb = const.tile([COUT, 1], F32)
        nc.sync.dma_start(chs_sb[:], chs_d[:])
        chb_sb = const.tile([COUT, 1], F32)
        nc.sync.dma_start(chb_sb[:], chb_d[:])
        av_sb = const.tile([CIN, 2], F32R)
        nc.scalar.dma_start(av_sb[:], av_d[:].bitcast(F32R))
        band_sb = const.tile([BK, 6 * 128], F32R)
        nc.scalar.dma_start(band_sb[:], band_d[:].bitcast(F32R))
        # map4 staging tiles (partitions 0/32/64/96 hold map values); memset
        # once so stream_shuffle never reads uninitialized SBUF.
        map4 = [const.tile([128, BW * W], F32, name="map4_0")]
        nc.gpsimd.memset(map4[0][:], 0.0)

        x_t = [None] * NXT
        FB = [None] * 9              # 16-f-row block -> SBUF f tile
        att_sb = [None] * NGRP       # group -> SBUF att tile [2, n*W]
        att_rt = [None] * NBLK       # block -> row-layout att tile
        att_rt_dmas = [[] for _ in range(NBLK)]
        pending_out = []             # deferred HWDGE output pieces
        blocks_done = [False] * NBLK
        map4_idx = 0

        def emit_att_rt_dma(g):
            """Repack att_sb[g] rows into row-layout block tiles."""
            f0, n = _grp_rows(g)
            for b, (o0, bw) in enumerate(BLOCKS):
                blo, bhi = o0 - 1, o0 + bw + 1
                lo, hi = max(f0, blo), min(f0 + n, bhi)
                if lo >= hi:
                    continue
                if att_rt[b] is None:
                    att_rt[b] = rtp.tile([BK, 2 * WP], F32R, name="att_rt")
                    # zero the x-pad columns (cols 0 and 257 of each branch)
                    for ci, c in enumerate((0, WP - 1, WP, 2 * WP - 1)):
                        att_rt_dmas[b].append(nc.gpsimd.dma_start(
                            att_rt[b][:, c:c + 1],
                            zeros_d[:, ci:ci + 1].bitcast(F32R)))
                    # pos2 rows (matched by identity rows in the dx=1 band)
                    for br in range(2):
                        att_rt_dmas[b].append(nc.gpsimd.dma_start(
                            att_rt[b][bw + 2:2 * bw + 2,
                                      br * WP + 1:br * WP + 1 + W],
                            pos2_d[o0 - 1:o0 - 1 + bw, br, :].bitcast(F32R)))
                for br in range(2):
                    att_rt_dmas[b].append(nc.sync.dma_start(
                        att_rt[b][lo - blo:hi - blo, br * WP + 1:br * WP + 1 + W],
                        att_sb[g][br:br + 1, (lo - f0) * W:(hi - f0) * W],
                    ))

        def emit_block(b):
            """Banded 3x3 attention conv + sigmoid + map broadcast + output
            multiplies for block b."""
            nonlocal map4_idx
            o0, bw = BLOCKS[b]
            kk = 2 * bw + 2
            z2a = z2ps.tile([BW, W], F32, tag="z2", name="z2a")
            z2b = z2ps.tile([BW, W], F32, tag="z2", name="z2b")
            for br, z2 in ((0, z2a), (1, z2b)):
                for dx in range(3):
                    cix = _band_off(b) + (br * 3 + dx) * bw
                    nc.tensor.matmul(
                        z2[:bw, :], band_sb[0:kk, cix:cix + bw],
                        att_rt[b][0:kk, br * WP + dx:br * WP + dx + W],
                        start=(dx == 0), stop=(dx == 2))
            mapS = mapp.tile([BW, 2 * W], F32, name="mapS")
            nc.scalar.activation(mapS[:bw, 0:W], z2a[:bw, :], AF.Sigmoid)
            nc.scalar.activation(mapS[:bw, W:2 * W], z2b[:bw, :], AF.Sigmoid)
            # broadcast: rows of this block to channel partitions 0/32/64/96
            m4 = map4[0]
            for br, parts in ((0, (0, 32)), (1, (64, 96))):
                src = mapS[:bw, br * W:(br + 1) * W]
                for p in parts:
                    nc.scalar.dma_start(m4[p:p + 1, :bw * W], src)
            mrep = mapp.tile([128, BW * W], F32, name="mrep", bufs=2)
            nc.vector.stream_shuffle(mrep[:, :bw * W], m4[:, :bw * W], [0] * 32)
            flush_pending_out()
            # multiply per overlapping 16-row f tile; first piece goes out
            # via SWDGE now, the rest via sync/HWDGE one block later (their
            # muls are then long done, so the sync queue never stalls)
            first = True
            for beta in range(o0 // 16, (o0 + bw - 1) // 16 + 1):
                lo, hi = max(o0, 16 * beta), min(o0 + bw, 16 * beta + 16)
                if lo >= hi:
                    continue
                dst = FB[beta][:, (lo - 16 * beta) * W:(hi - 16 * beta) * W]
                nc.vector.tensor_mul(
                    dst, dst, mrep[:, (lo - o0) * W:(hi - o0) * W])
                if first:
                    cut = lo + (hi - lo) // 4 if hi - lo > 3 else hi
                    nc.gpsimd.dma_start(
                        out_d[:, lo - 1:cut - 1, :],
                        dst[:, 0:(cut - lo) * W].bitcast(F32))
                    first = False
                else:
                    cut = lo
                if cut < hi:
                    pending_out.append((out_d[:, cut - 1:hi - 1, :],
                                        dst[:, (cut - lo) * W:].bitcast(F32)))
            if b == NBLK - 1:
                flush_pending_out()
            blocks_done[b] = True

        def flush_pending_out():
            while pending_out:
                o, src = pending_out.pop(0)
                nc.sync.dma_start(o, src)

        def emit_att(g):
            """M=2 attention matmul over f rows of group g + evac to att_sb."""
            f0, n = _grp_rows(g)
            beta = f0 // 16
            off = (f0 - 16 * beta) * W
            att_ps = atps.tile([2, G * W], F32, name="att_ps")
            for c in range(n * W // 512):
                nc.tensor.matmul(
                    att_ps[:, c * 512:(c + 1) * 512], av_sb[:],
                    FB[beta][:, off + c * 512:off + (c + 1) * 512],
                    start=True, stop=True)
            att_sb[g] = attp.tile([2, G * W], F32R, name="att_sb")
            nc.scalar.copy(att_sb[g][0:2, :n * W], att_ps[:, :n * W])
            emit_att_rt_dma(g)

        def flush_ready(last_att_g):
            """Emit any blocks whose att inputs are now all repacked."""
            att_hi = G * last_att_g + _grp_rows(last_att_g)[1]
            for b, (o0, bw) in enumerate(BLOCKS):
                if not blocks_done[b] and att_hi >= o0 + bw + 1:
                    emit_block(b)

        for g in range(NGRP):
            f0, n = _grp_rows(g)
            k = g // 3
            if x_t[k] is None:
                nrows = min(XROWS + 2, 132 - XROWS * k)
                x_t[k] = xp.tile([CIN, nrows, WP], F32R, name="x_t")
                third = (nrows + 2) // 3
                cuts = ([0, 6, 10, nrows] if k == 0
                        else [0, third, 2 * third, nrows])
                for c0, c1 in zip(cuts[:-1], cuts[1:]):
                    nc.sync.dma_start(
                        x_t[k][:, c0:c1, :],
                        xh_d[:, XROWS * k + c0:XROWS * k + c1, :].bitcast(F32R))
            xr0 = f0 - XROWS * k  # group's first f row within the x tile
            # conv: 9 taps x row-pair matmuls
            f_ps = fps.tile([COUT, G * W], F32, tag="big", name="f_ps")
            for tap in range(9):
                ky, kx = tap // 3, tap % 3
                lhsT = wf_sb[:, tap * COUT:(tap + 1) * COUT]
                for j in range(n // 2):
                    rhs = x_t[k][:, xr0 + 2 * j + ky:xr0 + 2 * j + ky + 2,
                                 kx:kx + W]
                    nc.tensor.matmul(
                        f_ps[:, j * 512:(j + 1) * 512], lhsT,
                        rhs, start=(tap == 0), stop=(tap == 8))
            # evac with fused per-channel scale+bias into the 16-row f tile
            beta = f0 // 16
            if FB[beta] is None:
                FB[beta] = fbp.tile([COUT, 16 * W], F32R, name="FB")
            off = (f0 - 16 * beta) * W
            nc.scalar.activation(FB[beta][:, off:off + n * W], f_ps[:, :n * W],
                                 AF.Identity, bias=chb_sb[:], scale=chs_sb[:])
            # attention matmul two groups back (its evac is surely done)
            if g >= 2:
                emit_att(g - 2)
                flush_ready(g - 2)
        emit_att(NGRP - 2)
        emit_att(NGRP - 1)
        flush_ready(NGRP - 1)
        flush_pending_out()
        assert all(blocks_done)


# ---------------------------------------------------------------- host side --

def _position_grids():
    i = np.arange(H, dtype=np.float64)
    j = np.arange(W, dtype=np.float64)
    gh = np.abs(i - H // 2 + 0.5) / float(H // 2)
    gw = np.abs(j - W // 2 + 0.5) / float(W // 2)
    GH = np.broadcast_to(gh[:, None], (H, W))
    GW = np.broadcast_to(gw[None, :], (H, W))
    pr = np.sqrt(GH ** 2 + GW ** 2)
    k = 2.0 / (pr.max() - pr.min())
    pr = k * pr + (1.0 - pr.max() * k)
    return GH, GW, pr


def _conv3x3_zp(x, w):
    """x: [C, H, W], w: [O, C, 3, 3] -> [O, H, W] zero-padded conv."""
    C, H_, W_ = x.shape
    O = w.shape[0]
    xp = np.pad(x, ((0, 0), (1, 1), (1, 1)))
    out = np.zeros((O, H_, W_), np.float64)
    for ky in range(3):
        for kx in range(3):
            out += np.einsum("oc,chw->ohw", w[:, :, ky, kx],
                             xp[:, ky:ky + H_, kx:kx + W_])
    return out


def fold_inputs(inp):
    """Host-side constant folding. Returns (shared constants, per-half consts)."""
    gh, gw, pr = _position_grids()
    Wf = np.zeros((COUT, CIN, 3, 3), np.float64)
    bf = np.zeros(COUT, np.float64)
    A = np.zeros((CIN, 2), np.float64)
    pos2 = np.zeros((2, H, W), np.float64)
    scales = np.array([float(np.asarray(inp["scale1"])),
                       float(np.asarray(inp["scale2"]))])
    bandw = np.zeros((2, 3, 3), np.float64)
    for bi, br in enumerate("ab"):
        k1 = np.asarray(inp[f"bn_{br}_gamma"], np.float64) / np.sqrt(
            np.asarray(inp[f"bn_{br}_var"], np.float64) + EPS_BR)
        Wf[bi * BR:(bi + 1) * BR] = (
            np.asarray(inp[f"conv_{br}_w"], np.float64) * k1[:, None, None, None])
        bf[bi * BR:(bi + 1) * BR] = (
            (np.asarray(inp[f"conv_{br}_b"], np.float64)
             - np.asarray(inp[f"bn_{br}_mean"], np.float64)) * k1
            + np.asarray(inp[f"bn_{br}_beta"], np.float64))
        k2 = (float(np.asarray(inp[f"att_bn_{br}_gamma"])[0])
              / np.sqrt(float(np.asarray(inp[f"att_bn_{br}_var"])[0]) + EPS_ATT))
        wa = np.asarray(inp[f"att_{br}_w"], np.float64)[0, :, 0, 0]
        s = scales[bi]
        A[bi * BR:(bi + 1) * BR, bi] = (wa[:BR] * k2 / s) if s != 0.0 else 0.0
        pos1 = (k2 * (wa[BR] * gh + wa[BR + 1] * gw
                      + float(np.asarray(inp[f"att_{br}_b"])[0])
                      - float(np.asarray(inp[f"att_bn_{br}_mean"])[0]))
                + float(np.asarray(inp[f"att_bn_{br}_beta"])[0]))
        attn_w = np.asarray(inp[f"attn_{br}_w"], np.float64)
        pos2[bi] = _conv3x3_zp(np.stack([pos1, gh, gw, pr]), attn_w)[0]
        bandw[bi] = attn_w[0, 0]
    ch_scale = np.repeat(scales, BR)
    shared = {
        # wf DRAM layout: [cin, tap, cout]
        "wf": np.ascontiguousarray(
            Wf.transpose(1, 2, 3, 0).reshape(CIN, 9 * COUT)).astype(np.float32),
        "chs": ch_scale.reshape(COUT, 1).astype(np.float32),
        "chb": (bf * ch_scale).reshape(COUT, 1).astype(np.float32),
        "av": A.astype(np.float32),
    }
    halves = []
    for half in range(2):
        r0 = half * 128
        band = np.zeros((BK, 6 * 128), np.float64)
        for b, (o0, bw) in enumerate(BLOCKS):
            base = _band_off(b)
            for i in range(bw + 2):
                fr_in = o0 - 1 + i
                absr = r0 - 1 + fr_in
                if not (0 <= absr < H):
                    continue
                for o in range(bw):
                    dy = i - o
                    if 0 <= dy <= 2:
                        for br in range(2):
                            band[i, base + br * 3 * bw:base + (br * 3 + 3) * bw]
                            for dx in range(3):
                                band[i, base + (br * 3 + dx) * bw + o] = \
                                    bandw[br, dy, dx]
            for o in range(bw):
                # identity rows add pos2 (staged in att_rt) at dx=1
                for br in range(2):
                    band[bw + 2 + o, base + (br * 3 + 1) * bw + o] = 1.0
        p2 = np.zeros((128, 2, W), np.float64)
        for ro in range(128):
            p2[ro] = pos2[:, r0 + ro]
        halves.append({
            "band": np.ascontiguousarray(band).astype(np.float32),
            "pos2": p2.astype(np.float32),
        })
    return shared, halves


def make_in_maps(inp):
    shared, halves = fold_inputs(inp)
    x = np.asarray(inp["x"], np.float32)
    in_maps = []
    for core in range(8):
        b, half = core // 2, core % 2
        r0 = half * 128
        xpad = np.pad(x[b], ((0, 0), (2, 2), (1, 1)))
        xh = np.ascontiguousarray(xpad[:, r0:r0 + 132, :])
        in_maps.append({"xh": xh, "zeros": np.zeros((BK, 4), np.float32),
                        **shared, **halves[half]})
    return in_maps


def _split_matmul_waits(nc):
    """This walrus build accepts only ONE sync wait command per engine
    instruction struct. Move extra waits onto sequencer NoOps inserted just
    before the instruction: the engine queue is processed in order, so the
    sequencer blocks on the NoOp's waits before dispatching it."""
    cnt = 0
    for fn in nc.m.functions:
        for bb in fn.blocks:
            insts = bb.instructions
            i = 0
            while i < len(insts):
                ins = insts[i]
                if (not isinstance(ins, mybir.InstNoOp) and ins.is_executable()
                        and ins.sync_info is not None):
                    w = list(ins.sync_info.on_wait)
                    if len(w) > 1:
                        ins.sync_info = mybir.SyncInfo(
                            on_wait=[w[0]],
                            on_update=list(ins.sync_info.on_update))
                        for sw in w[1:]:
                            cnt += 1
                            nop = mybir.InstNoOp(
                                name=f"I-mmwait-{cnt}", ins=[], outs=[])
                            nop.engine = ins.engine
                            nop.sync_info = mybir.SyncInfo(
                                on_wait=[sw], on_update=[])
                            insts.insert(i, nop)
                            i += 1
                i += 1
    return cnt


_PROGRAM = None


def _build_program():
    global _PROGRAM
    if _PROGRAM is not None:
        return _PROGRAM
    from concourse._compat import axon_active
    nc = bass.Bass("TRN2", target_bir_lowering=False,
                   debug=not axon_active(), enable_asserts=False,
                   num_devices=8)
    ins = {
        "xh": nc.dram_tensor("xh", [CIN, 132, WP], F32, kind="ExternalInput").ap(),
        "wf": nc.dram_tensor("wf", [CIN, 9 * COUT], F32, kind="ExternalInput").ap(),
        "chs": nc.dram_tensor("chs", [COUT, 1], F32, kind="ExternalInput").ap(),
        "chb": nc.dram_tensor("chb", [COUT, 1], F32, kind="ExternalInput").ap(),
        "av": nc.dram_tensor("av", [CIN, 2], F32, kind="ExternalInput").ap(),
        "band": nc.dram_tensor("band", [BK, 6 * 128], F32,
                               kind="ExternalInput").ap(),
        "pos2": nc.dram_tensor("pos2", [128, 2, W], F32,
                               kind="ExternalInput").ap(),
        "zeros": nc.dram_tensor("zeros", [BK, 4], F32,
                                kind="ExternalInput").ap(),
    }
    outs = {
        "out": nc.dram_tensor("out", [COUT, 128, W], F32,
                              kind="ExternalOutput").ap(),
    }
    with tile.TileContext(nc) as tc:
        emit_core(tc, outs, ins)
    _split_matmul_waits(nc)
    _PROGRAM = nc
    return nc


def run_cores(inp, trace=False, **kw):
    """Run the SPMD kernel; returns (full output, BassKernelResults)."""
    nc = _build_program()
    in_maps = make_in_maps(inp)
    res = run_bass_kernel_spmd(nc, in_maps, core_ids=list(range(8)),
                               trace=trace, **kw)
    out = np.zeros((B, COUT, H, W), np.float32)
    for core in range(8):
        b, half = core // 2, core % 2
        out[b, :, half * 128:half * 128 + 128] = res.results[core]["out"]
    return out, res


def kernel(**inputs):
    out, _ = run_cores(inputs)
    return out
